# revision 14
# baseline (speedup 1.0000x reference)
"""Bahdanau-attention kernel for 8 Trainium2 NeuronCores.

Math: reference computes
    energy = cat([hidden, eo], 1) @ attn_w.T + attn_b      # [S, H]
    scores = energy @ other[0]                             # [S]
    attn   = softmax(scores)
Because softmax is shift-invariant, the contributions of `hidden` and
`attn_b` (constant across the sequence axis) cancel, leaving
    attn = softmax(eo @ v),   v = attn_w[:, H:].T @ other[0]
which is two mat-vecs instead of an [S,2H]x[2H,H] matmul. The kernel is
memory-bound: it reads eo (128 MB) and W2 = attn_w[:, H:] (64 MB) once.

Sharding (8 cores): both eo and W2 are sharded along the *hidden* axis
(columns). Core k holds eo[:, 512k:512k+512] and attn_w[:, H+512k:...],
computes its 512 elements of v locally (no communication), then partial
scores for ALL of S over its columns. One AllReduce of the [S] partial
scores at the very end combines them; every core then computes the
(identical) softmax and core 0's output is used. The single collective
sits at the end so the ~80us first-collective/ncfw-boot latency of this
runtime overlaps the DMA + compute phase.

Host-side prep pre-swizzles each shard into the exact SBUF image so
every DMA line is 16 KB contiguous (2 KB lines measured ~2.5x slower).
"""

import os
import sys

import numpy as np

for _p in ("/opt/trn_rl_repo",):
    if os.path.isdir(_p) and _p not in sys.path:
        sys.path.insert(0, _p)

import concourse.bacc as bacc
import concourse.bass as bass
import concourse.masks as masks
import concourse.mybir as mybir
import concourse.tile as tile
from concourse.bass_utils import run_bass_kernel_spmd
from concourse.tile_rust import add_dep_helper

H = 4096
S = 8192
NCORES = 8
I_SH = H // NCORES      # 512 hidden columns per core
F32 = mybir.dt.float32
F32R = mybir.dt.float32r

# Results of the most recent run (profiling info etc), for test harnesses.
LAST_RESULT = None

_MODULE_CACHE = None


def _build_module():
    nc = bacc.Bacc(
        "TRN2",
        target_bir_lowering=False,
        debug=False,
        enable_asserts=False,
        num_devices=NCORES,
    )

    # eo_img[p, n, i] = eo[128n + p, 512k + i]  (host pre-swizzled)
    eo_in = nc.dram_tensor("eo_img", [128, S // 128, I_SH], F32,
                           kind="ExternalInput")
    # w2img[p, m, i] = attn_w[128m + p, H + 512k + i]; float32r lets the PE
    # run the v mat-vec at 1 cyc/row (fp32 is 4 cyc/row); ~1e-4 relative
    # error on v, irrelevant here (score gaps are ~20).
    w2_in = nc.dram_tensor("w2img", [128, H // 128, I_SH], F32R,
                           kind="ExternalInput")
    oth_in = nc.dram_tensor("other_t", [128, H // 128], F32R,
                            kind="ExternalInput")
    out_t = nc.dram_tensor("attn_out", [S], F32, kind="ExternalOutput")

    with tile.TileContext(nc) as tc:
        _kernel_body(tc, nc, eo_in, w2_in, oth_in, out_t)

    nc.compile()
    return nc


def _kernel_body(tc, nc, eo_in, w2_in, oth_in, out_t):
    RG = [list(range(NCORES))]
    Alu = mybir.AluOpType
    Act = mybir.ActivationFunctionType
    X = mybir.AxisListType.X
    NM = H // 128            # 32 contraction chunks for v
    NS = S // 128            # 64 sequence chunks
    NT = 8                   # eo DMA tiles (8 chunks each)
    CPT = NS // NT           # sequence chunks per eo tile
    NW = 4                   # W2 DMA waves
    MPW = NM // NW

    with (
        tc.tile_pool(name="const", bufs=1) as constp,
        tc.tile_pool(name="w2p", bufs=4) as w2p,
        tc.tile_pool(name="eop", bufs=6) as eop,
        tc.tile_pool(name="scrp", bufs=2) as scrp,
        tc.tile_pool(name="vp", bufs=1) as vp,
        tc.tile_pool(name="psp", bufs=2, space="PSUM") as psp,
        tc.tile_pool(name="dramp", bufs=1, space="DRAM") as dramp,
    ):
        # ---- warmup collective (prime ncfw while DMA/compute runs) -----
        warm_sb = constp.tile([1, 1], F32)
        nc.vector.memset(warm_sb[:], 0.0)
        warm_loc = dramp.tile([1], F32)
        nc.scalar.dma_start(warm_loc[None, :], warm_sb[:])
        warm_out = dramp.tile([NCORES], F32, addr_space="Shared")
        nc.gpsimd.collective_compute(
            "AllGather", Alu.bypass, replica_groups=RG,
            ins=[warm_loc[None, :]], outs=[warm_out[None, :]],
        )

        # ---- constants -------------------------------------------------
        ident = constp.tile([128, 128], F32)
        masks.make_identity(nc, ident[:])
        ones_row = constp.tile([1, 128], F32)
        nc.vector.memset(ones_row[:], 1.0)
        neg_row = constp.tile([1, 128], F32)
        nc.vector.memset(neg_row[:], -1.0)
        # Preload the exp table set early so the ~2.7us load overlaps DMA.
        dummy = constp.tile([1, 1], F32)
        nc.vector.memset(dummy[:], 0.0)
        nc.scalar.activation(dummy[:], dummy[:], Act.Exp)

        oth_sb = constp.tile([128, NM], F32R)
        nc.scalar.dma_start(oth_sb[:], oth_in[:, :])

        # ---- local v chunk: v[512k:512k+512] on the PE -----------------
        v_ps = psp.tile([1, I_SH], F32, tag="vps", bufs=1)
        w2_dmas = []
        for c in range(NW):
            w2_t = w2p.tile([128, MPW, I_SH], F32R, tag="w2")
            w2_dmas.append(
                nc.sync.dma_start(w2_t[:], w2_in[:, c * MPW:(c + 1) * MPW, :])
            )
            for j in range(MPW):
                m = c * MPW + j
                nc.tensor.matmul(
                    v_ps[:],
                    lhsT=oth_sb[:, m : m + 1],
                    rhs=w2_t[:, j, :],
                    start=(m == 0),
                    stop=(m == NM - 1),
                )
        v_loc_sb = vp.tile([1, I_SH], F32)
        nc.vector.tensor_copy(v_loc_sb[:], v_ps[:])

        # broadcast the local v chunk to all 128 partitions on-chip:
        # ones[128,1] (x) v[1,512] via one K=1 matmul (exact: weights are 1.0)
        bc_ps = psp.tile([128, I_SH], F32, tag="bcps", bufs=1)
        nc.tensor.matmul(bc_ps[:], lhsT=ones_row[:], rhs=v_loc_sb[:],
                         start=True, stop=True)
        v_bc = vp.tile([128, I_SH], F32)
        nc.vector.tensor_copy(v_bc[:], bc_ps[:])

        # ---- partial scores for ALL of S over my 512 columns -----------
        scores_sb = vp.tile([128, NS], F32)
        first_eo_dma = None
        sc_loc_dram = dramp.tile([S], F32)
        sc_dram_a = dramp.tile([S // 2], F32, addr_space="Shared")
        sc_dram_b = dramp.tile([S // 2], F32, addr_space="Shared")
        sc_halves = [sc_dram_a, sc_dram_b]
        sc_loc_view = sc_loc_dram.rearrange("(n p) -> n p", p=128)

        def _reduce_half(h):
            """Transpose scores chunks [32h, 32h+32) to s-order and
            AllReduce that half. Half 0 fires mid-STT so the cross-core
            rendezvous overlaps the remaining DVE work."""
            tr_ps = psp.tile([NS // 2, 128], F32, tag="tp", bufs=2,
                             name=f"tr_ps{h}")
            nc.tensor.matmul(
                tr_ps[:], lhsT=scores_sb[:, h * NS // 2:(h + 1) * NS // 2],
                rhs=ident[:], is_transpose=True, start=True, stop=True,
            )
            tr_sb = vp.tile([NS // 2, 128], F32, name=f"tr_sb{h}")
            nc.scalar.copy(tr_sb[:], tr_ps[:])
            nc.scalar.dma_start(
                sc_loc_view[h * NS // 2:(h + 1) * NS // 2, :], tr_sb[:]
            )
            nc.gpsimd.collective_compute(
                "AllReduce", Alu.add, replica_groups=RG,
                ins=[sc_loc_dram[None, h * S // 2:(h + 1) * S // 2]],
                outs=[sc_halves[h][None, :]],
            )

        for t in range(NT):
            eo_t = eop.tile([128, CPT, I_SH], F32, tag="eo")
            dma = nc.sync.dma_start(
                eo_t[:], eo_in[:, t * CPT:(t + 1) * CPT, :]
            )
            if t == 0:
                first_eo_dma = dma
            for c in range(CPT):
                scratch = scrp.tile([128, I_SH], F32, tag="ttr")
                # out = (eo * 1.0) * v ; accum_out = sum(out): fused
                # multiply+reduce (tensor_tensor_reduce crashes here).
                nc.vector.scalar_tensor_tensor(
                    out=scratch[:],
                    in0=eo_t[:, c, :],
                    scalar=1.0,
                    in1=v_bc[:],
                    op0=Alu.mult,
                    op1=Alu.mult,
                    accum_out=scores_sb[:, t * CPT + c : t * CPT + c + 1],
                )
            if t == NT // 2 - 1:
                _reduce_half(0)
        _reduce_half(1)
        # keep the eo stream from stealing SDMA bandwidth from W2 (the
        # critical path for v)
        add_dep_helper(
            first_eo_dma.ins, w2_dmas[-1].ins, sync=True,
            reason="serialize eo stream behind W2 (critical path)",
        )

        # ---- softmax over all S scores (replicated on every core) ------
        # s = 64p + c, so half A (s < 4096) is exactly partitions 0..63
        sm_sb = vp.tile([128, S // 128], F32)
        nc.scalar.dma_start(sm_sb[0:64, :],
                            sc_dram_a.rearrange("(p c) -> p c", p=64))
        nc.scalar.dma_start(sm_sb[64:128, :],
                            sc_dram_b.rearrange("(p c) -> p c", p=64))

        m1 = vp.tile([128, 1], F32)
        nc.vector.tensor_reduce(m1[:], sm_sb[:], X, Alu.max)
        m1t_ps = psp.tile([1, 128], F32, tag="tp", bufs=2)
        nc.tensor.matmul(m1t_ps[:], lhsT=m1[:], rhs=ident[:],
                         is_transpose=True, start=True, stop=True)
        m1t_sb = vp.tile([1, 128], F32)
        nc.scalar.copy(m1t_sb[:], m1t_ps[:])
        gmax = vp.tile([1, 1], F32)
        nc.vector.tensor_reduce(gmax[:], m1t_sb[:], X, Alu.max)

        negmax_ps = psp.tile([128, 1], F32, tag="tp", bufs=2)
        nc.tensor.matmul(negmax_ps[:], lhsT=neg_row[:], rhs=gmax[:],
                         start=True, stop=True)
        negmax_sb = vp.tile([128, 1], F32)
        nc.scalar.copy(negmax_sb[:], negmax_ps[:])

        probs = vp.tile([128, S // 128], F32)
        sumexp = vp.tile([128, 1], F32)
        nc.scalar.activation(probs[:], sm_sb[:], Act.Exp, bias=negmax_sb[:],
                             scale=1.0, accum_out=sumexp[:])

        set_ps = psp.tile([1, 128], F32, tag="tp", bufs=2)
        nc.tensor.matmul(set_ps[:], lhsT=sumexp[:], rhs=ident[:],
                         is_transpose=True, start=True, stop=True)
        se_sb = vp.tile([1, 128], F32)
        nc.scalar.copy(se_sb[:], set_ps[:])
        ssum = vp.tile([1, 1], F32)
        nc.vector.tensor_reduce(ssum[:], se_sb[:], X, Alu.add)
        rinv = vp.tile([1, 1], F32)
        nc.vector.reciprocal(rinv[:], ssum[:])
        rinv_ps = psp.tile([128, 1], F32, tag="tp", bufs=2)
        nc.tensor.matmul(rinv_ps[:], lhsT=ones_row[:], rhs=rinv[:],
                         start=True, stop=True)
        rinv_sb = vp.tile([128, 1], F32)
        nc.scalar.copy(rinv_sb[:], rinv_ps[:])

        attn_sb = vp.tile([128, S // 128], F32)
        nc.vector.tensor_scalar_mul(attn_sb[:], probs[:], rinv_sb[:])
        nc.scalar.dma_start(out_t.rearrange("(p c) -> p c", p=128), attn_sb[:])


def _get_module():
    global _MODULE_CACHE
    if _MODULE_CACHE is None:
        _MODULE_CACHE = _build_module()
    return _MODULE_CACHE


def kernel(hidden, encoder_outputs, attn_w, attn_b, other):
    """Full inputs in, full output out; distributes across 8 NeuronCores."""
    global LAST_RESULT
    eo = np.asarray(encoder_outputs, dtype=np.float32).reshape(S, H)
    w = np.asarray(attn_w, dtype=np.float32)
    oth = np.asarray(other, dtype=np.float32).reshape(H)
    # hidden / attn_b shift all scores equally; softmax cancels them.

    oth_t = np.ascontiguousarray(oth.reshape(H // 128, 128).T)  # [128, 32]

    in_maps = []
    for k in range(NCORES):
        cols = slice(k * I_SH, (k + 1) * I_SH)
        # [128, 64, 512]: eo_img[p, n, i] = eo[128n + p, 512k + i]
        eo_img = np.ascontiguousarray(
            eo[:, cols].reshape(S // 128, 128, I_SH).transpose(1, 0, 2)
        )
        # [128, 32, 512]: w2img[p, m, i] = attn_w[128m + p, H + 512k + i]
        w2_img = np.ascontiguousarray(
            w[:, H + k * I_SH : H + (k + 1) * I_SH]
            .reshape(H // 128, 128, I_SH)
            .transpose(1, 0, 2)
        )
        in_maps.append(
            {"eo_img": eo_img, "w2img": w2_img, "other_t": oth_t}
        )

    nc = _get_module()
    LAST_RESULT = run_bass_kernel_spmd(
        nc,
        in_maps,
        core_ids=list(range(NCORES)),
    )
    out = np.asarray(LAST_RESULT.results[0]["attn_out"], dtype=np.float32)
    return out.reshape(1, 1, S)


if __name__ == "__main__":
    rng = np.random.default_rng(0)
    inputs = {
        "hidden": rng.standard_normal((1, H), dtype=np.float32),
        "encoder_outputs": rng.standard_normal((S, 1, H), dtype=np.float32),
        "attn_w": (rng.standard_normal((H, 2 * H), dtype=np.float32)
                   / np.sqrt(2 * H)).astype(np.float32),
        "attn_b": (rng.standard_normal(H, dtype=np.float32)
                   / np.sqrt(2 * H)).astype(np.float32),
        "other": rng.standard_normal((1, H), dtype=np.float32),
    }
    out = kernel(**inputs)
    print("out", out.shape, out.dtype, out.sum())


# revision 15
# speedup vs baseline: 1.1180x; 1.1180x over previous
"""Bahdanau-attention kernel for 8 Trainium2 NeuronCores.

Math: reference computes
    energy = cat([hidden, eo], 1) @ attn_w.T + attn_b      # [S, H]
    scores = energy @ other[0]                             # [S]
    attn   = softmax(scores)
Because softmax is shift-invariant, the contributions of `hidden` and
`attn_b` (constant across the sequence axis) cancel, leaving
    attn = softmax(eo @ v),   v = attn_w[:, H:].T @ other[0]
which is two mat-vecs instead of an [S,2H]x[2H,H] matmul. The kernel is
memory-bound: it reads eo (128 MB) and W2 = attn_w[:, H:] (64 MB) once.

Sharding (8 cores): both eo and W2 are sharded along the *hidden* axis
(columns). Core k holds eo[:, 512k:512k+512] and attn_w[:, H+512k:...],
computes its 512 elements of v locally (no communication), then partial
scores for ALL of S over its columns. One AllReduce of the [S] partial
scores at the very end combines them; every core then computes the
(identical) softmax and core 0's output is used. The single collective
sits at the end so the ~80us first-collective/ncfw-boot latency of this
runtime overlaps the DMA + compute phase.

Host-side prep pre-swizzles each shard into the exact SBUF image so
every DMA line is 16 KB contiguous (2 KB lines measured ~2.5x slower).
"""

import os
import sys

import numpy as np

for _p in ("/opt/trn_rl_repo",):
    if os.path.isdir(_p) and _p not in sys.path:
        sys.path.insert(0, _p)

import concourse.bacc as bacc
import concourse.bass as bass
import concourse.masks as masks
import concourse.mybir as mybir
import concourse.tile as tile
from concourse.bass_utils import run_bass_kernel_spmd
from concourse.tile_rust import add_dep_helper

H = 4096
S = 8192
NCORES = 8
I_SH = H // NCORES      # 512 hidden columns per core
F32 = mybir.dt.float32
F32R = mybir.dt.float32r

# Results of the most recent run (profiling info etc), for test harnesses.
LAST_RESULT = None

_MODULE_CACHE = None


def _build_module():
    nc = bacc.Bacc(
        "TRN2",
        target_bir_lowering=False,
        debug=False,
        enable_asserts=False,
        num_devices=NCORES,
    )

    # eo_img[p, n, i] = eo[128n + p, 512k + i]  (host pre-swizzled)
    eo_in = nc.dram_tensor("eo_img", [128, S // 128, I_SH], F32,
                           kind="ExternalInput")
    # w2img[p, m, i] = attn_w[128m + p, H + 512k + i]; float32r lets the PE
    # run the v mat-vec at 1 cyc/row (fp32 is 4 cyc/row); ~1e-4 relative
    # error on v, irrelevant here (score gaps are ~20).
    w2_in = nc.dram_tensor("w2img", [128, H // 128, I_SH], F32R,
                           kind="ExternalInput")
    oth_in = nc.dram_tensor("other_t", [128, H // 128], F32R,
                            kind="ExternalInput")
    out_t = nc.dram_tensor("attn_out", [S], F32, kind="ExternalOutput")

    with tile.TileContext(nc) as tc:
        _kernel_body(tc, nc, eo_in, w2_in, oth_in, out_t)

    nc.compile()
    return nc


def _kernel_body(tc, nc, eo_in, w2_in, oth_in, out_t):
    RG = [list(range(NCORES))]
    Alu = mybir.AluOpType
    Act = mybir.ActivationFunctionType
    X = mybir.AxisListType.X
    NM = H // 128            # 32 contraction chunks for v
    NS = S // 128            # 64 sequence chunks
    NT = 8                   # eo DMA tiles (8 chunks each)
    CPT = NS // NT           # sequence chunks per eo tile
    NW = 4                   # W2 DMA waves
    MPW = NM // NW

    with (
        tc.tile_pool(name="const", bufs=1) as constp,
        tc.tile_pool(name="w2p", bufs=4) as w2p,
        tc.tile_pool(name="eop", bufs=7) as eop,
        tc.tile_pool(name="scrp", bufs=2) as scrp,
        tc.tile_pool(name="vp", bufs=1) as vp,
        tc.tile_pool(name="psp", bufs=2, space="PSUM") as psp,
        tc.tile_pool(name="dramp", bufs=1, space="DRAM") as dramp,
    ):
        # ---- warmup collective (prime ncfw while DMA/compute runs) -----
        warm_sb = constp.tile([1, 1], F32)
        nc.vector.memset(warm_sb[:], 0.0)
        warm_loc = dramp.tile([1], F32)
        nc.scalar.dma_start(warm_loc[None, :], warm_sb[:])
        warm_out = dramp.tile([NCORES], F32, addr_space="Shared")
        nc.gpsimd.collective_compute(
            "AllGather", Alu.bypass, replica_groups=RG,
            ins=[warm_loc[None, :]], outs=[warm_out[None, :]],
        )

        # ---- constants -------------------------------------------------
        ident = constp.tile([128, 128], F32)
        masks.make_identity(nc, ident[:])
        ones_row = constp.tile([1, 128], F32)
        nc.vector.memset(ones_row[:], 1.0)
        neg_row = constp.tile([1, 128], F32)
        nc.vector.memset(neg_row[:], -1.0)
        # Preload the exp table set early so the ~2.7us load overlaps DMA.
        dummy = constp.tile([1, 1], F32)
        nc.vector.memset(dummy[:], 0.0)
        nc.scalar.activation(dummy[:], dummy[:], Act.Exp)

        oth_sb = constp.tile([128, NM], F32R)
        nc.scalar.dma_start(oth_sb[:], oth_in[:, :])

        # ---- local v chunk: v[512k:512k+512] on the PE -----------------
        v_ps = psp.tile([1, I_SH], F32, tag="vps", bufs=1)
        w2_dmas = []
        for c in range(NW):
            w2_t = w2p.tile([128, MPW, I_SH], F32R, tag="w2")
            w2_dmas.append(
                nc.sync.dma_start(w2_t[:], w2_in[:, c * MPW:(c + 1) * MPW, :])
            )
            for j in range(MPW):
                m = c * MPW + j
                nc.tensor.matmul(
                    v_ps[:],
                    lhsT=oth_sb[:, m : m + 1],
                    rhs=w2_t[:, j, :],
                    start=(m == 0),
                    stop=(m == NM - 1),
                )
        v_loc_sb = vp.tile([1, I_SH], F32)
        nc.vector.tensor_copy(v_loc_sb[:], v_ps[:])

        # broadcast the local v chunk to all 128 partitions on-chip:
        # ones[128,1] (x) v[1,512] via one K=1 matmul (exact: weights are 1.0)
        bc_ps = psp.tile([128, I_SH], F32, tag="bcps", bufs=1)
        nc.tensor.matmul(bc_ps[:], lhsT=ones_row[:], rhs=v_loc_sb[:],
                         start=True, stop=True)
        v_bc = vp.tile([128, I_SH], F32)
        nc.vector.tensor_copy(v_bc[:], bc_ps[:])

        # ---- partial scores for ALL of S over my 512 columns -----------
        scores_sb = vp.tile([128, NS], F32)
        first_eo_dma = None
        sc_loc_dram = dramp.tile([S], F32)
        sc_dram_a = dramp.tile([S // 2], F32, addr_space="Shared")
        sc_dram_b = dramp.tile([S // 2], F32, addr_space="Shared")
        sc_halves = [sc_dram_a, sc_dram_b]
        sc_loc_view = sc_loc_dram.rearrange("(n p) -> n p", p=128)

        def _reduce_half(h):
            """Transpose scores chunks [32h, 32h+32) to s-order and
            AllReduce that half. Half 0 fires mid-STT so the cross-core
            rendezvous overlaps the remaining DVE work."""
            tr_ps = psp.tile([NS // 2, 128], F32, tag="tp", bufs=2,
                             name=f"tr_ps{h}")
            nc.tensor.matmul(
                tr_ps[:], lhsT=scores_sb[:, h * NS // 2:(h + 1) * NS // 2],
                rhs=ident[:], is_transpose=True, start=True, stop=True,
            )
            tr_sb = vp.tile([NS // 2, 128], F32, name=f"tr_sb{h}")
            nc.scalar.copy(tr_sb[:], tr_ps[:])
            nc.scalar.dma_start(
                sc_loc_view[h * NS // 2:(h + 1) * NS // 2, :], tr_sb[:]
            )
            nc.gpsimd.collective_compute(
                "AllReduce", Alu.add, replica_groups=RG,
                ins=[sc_loc_dram[None, h * S // 2:(h + 1) * S // 2]],
                outs=[sc_halves[h][None, :]],
            )

        for t in range(NT):
            eo_t = eop.tile([128, CPT, I_SH], F32, tag="eo")
            dma = nc.sync.dma_start(
                eo_t[:], eo_in[:, t * CPT:(t + 1) * CPT, :]
            )
            if t == 0:
                first_eo_dma = dma
            for c in range(CPT):
                scratch = scrp.tile([128, I_SH], F32, tag="ttr")
                # out = (eo * 1.0) * v ; accum_out = sum(out): fused
                # multiply+reduce (tensor_tensor_reduce crashes here).
                nc.vector.scalar_tensor_tensor(
                    out=scratch[:],
                    in0=eo_t[:, c, :],
                    scalar=1.0,
                    in1=v_bc[:],
                    op0=Alu.mult,
                    op1=Alu.mult,
                    accum_out=scores_sb[:, t * CPT + c : t * CPT + c + 1],
                )
            if t == NT // 2 - 1:
                _reduce_half(0)
        _reduce_half(1)
        # keep the eo stream from stealing SDMA bandwidth from W2 (the
        # critical path for v)
        add_dep_helper(
            first_eo_dma.ins, w2_dmas[-2].ins, sync=True,
            reason="serialize eo stream behind most of W2 (critical path)",
        )

        # ---- softmax over all S scores (replicated on every core) ------
        # s = 64p + c, so half A (s < 4096) is exactly partitions 0..63
        sm_sb = vp.tile([128, S // 128], F32)
        nc.scalar.dma_start(sm_sb[0:64, :],
                            sc_dram_a.rearrange("(p c) -> p c", p=64))
        nc.scalar.dma_start(sm_sb[64:128, :],
                            sc_dram_b.rearrange("(p c) -> p c", p=64))

        m1 = vp.tile([128, 1], F32)
        nc.vector.tensor_reduce(m1[:], sm_sb[:], X, Alu.max)
        m1t_ps = psp.tile([1, 128], F32, tag="tp", bufs=2)
        nc.tensor.matmul(m1t_ps[:], lhsT=m1[:], rhs=ident[:],
                         is_transpose=True, start=True, stop=True)
        m1t_sb = vp.tile([1, 128], F32)
        nc.scalar.copy(m1t_sb[:], m1t_ps[:])
        gmax = vp.tile([1, 1], F32)
        nc.vector.tensor_reduce(gmax[:], m1t_sb[:], X, Alu.max)

        negmax_ps = psp.tile([128, 1], F32, tag="tp", bufs=2)
        nc.tensor.matmul(negmax_ps[:], lhsT=neg_row[:], rhs=gmax[:],
                         start=True, stop=True)
        negmax_sb = vp.tile([128, 1], F32)
        nc.scalar.copy(negmax_sb[:], negmax_ps[:])

        probs = vp.tile([128, S // 128], F32)
        sumexp = vp.tile([128, 1], F32)
        nc.scalar.activation(probs[:], sm_sb[:], Act.Exp, bias=negmax_sb[:],
                             scale=1.0, accum_out=sumexp[:])

        set_ps = psp.tile([1, 128], F32, tag="tp", bufs=2)
        nc.tensor.matmul(set_ps[:], lhsT=sumexp[:], rhs=ident[:],
                         is_transpose=True, start=True, stop=True)
        se_sb = vp.tile([1, 128], F32)
        nc.scalar.copy(se_sb[:], set_ps[:])
        ssum = vp.tile([1, 1], F32)
        nc.vector.tensor_reduce(ssum[:], se_sb[:], X, Alu.add)
        rinv = vp.tile([1, 1], F32)
        nc.vector.reciprocal(rinv[:], ssum[:])
        rinv_ps = psp.tile([128, 1], F32, tag="tp", bufs=2)
        nc.tensor.matmul(rinv_ps[:], lhsT=ones_row[:], rhs=rinv[:],
                         start=True, stop=True)
        rinv_sb = vp.tile([128, 1], F32)
        nc.scalar.copy(rinv_sb[:], rinv_ps[:])

        attn_sb = vp.tile([128, S // 128], F32)
        nc.vector.tensor_scalar_mul(attn_sb[:], probs[:], rinv_sb[:])
        nc.scalar.dma_start(out_t.rearrange("(p c) -> p c", p=128), attn_sb[:])


def _get_module():
    global _MODULE_CACHE
    if _MODULE_CACHE is None:
        _MODULE_CACHE = _build_module()
    return _MODULE_CACHE


def kernel(hidden, encoder_outputs, attn_w, attn_b, other):
    """Full inputs in, full output out; distributes across 8 NeuronCores."""
    global LAST_RESULT
    eo = np.asarray(encoder_outputs, dtype=np.float32).reshape(S, H)
    w = np.asarray(attn_w, dtype=np.float32)
    oth = np.asarray(other, dtype=np.float32).reshape(H)
    # hidden / attn_b shift all scores equally; softmax cancels them.

    oth_t = np.ascontiguousarray(oth.reshape(H // 128, 128).T)  # [128, 32]

    in_maps = []
    for k in range(NCORES):
        cols = slice(k * I_SH, (k + 1) * I_SH)
        # [128, 64, 512]: eo_img[p, n, i] = eo[128n + p, 512k + i]
        eo_img = np.ascontiguousarray(
            eo[:, cols].reshape(S // 128, 128, I_SH).transpose(1, 0, 2)
        )
        # [128, 32, 512]: w2img[p, m, i] = attn_w[128m + p, H + 512k + i]
        w2_img = np.ascontiguousarray(
            w[:, H + k * I_SH : H + (k + 1) * I_SH]
            .reshape(H // 128, 128, I_SH)
            .transpose(1, 0, 2)
        )
        in_maps.append(
            {"eo_img": eo_img, "w2img": w2_img, "other_t": oth_t}
        )

    nc = _get_module()
    LAST_RESULT = run_bass_kernel_spmd(
        nc,
        in_maps,
        core_ids=list(range(NCORES)),
    )
    out = np.asarray(LAST_RESULT.results[0]["attn_out"], dtype=np.float32)
    return out.reshape(1, 1, S)


if __name__ == "__main__":
    rng = np.random.default_rng(0)
    inputs = {
        "hidden": rng.standard_normal((1, H), dtype=np.float32),
        "encoder_outputs": rng.standard_normal((S, 1, H), dtype=np.float32),
        "attn_w": (rng.standard_normal((H, 2 * H), dtype=np.float32)
                   / np.sqrt(2 * H)).astype(np.float32),
        "attn_b": (rng.standard_normal(H, dtype=np.float32)
                   / np.sqrt(2 * H)).astype(np.float32),
        "other": rng.standard_normal((1, H), dtype=np.float32),
    }
    out = kernel(**inputs)
    print("out", out.shape, out.dtype, out.sum())
